# revision 1
# baseline (speedup 1.0000x reference)
"""DPA+SSM block kernel for 8 Trainium2 NeuronCores.

Sharding: data-parallel over the sequence (T=4096 -> 8 x 512 own tokens);
each core also receives a 256-token halo of the raw input before its own
range.  The attention window is 256, so the halo covers every key a core
needs; the SSM recurrence decay |A| < 0.1 makes state influence from before
the halo underflow fp32 entirely, so a zero-initialized scan warm-started
over the halo is exact.  No cross-core communication.

Layout: activations are feature-major [D, T] on the device (host transposes
in/out).  All linears run as W-chunk-stationary x activation-moving fp32r
matmuls.  LayerNorm statistics use an all-ones stationary matmul (partition
reduction, replicated over partitions).  The SSM scan is one exact DVE
tensor_tensor_scan.  V is produced token-major with an interleaved ones
column per head so each PV matmul also emits the softmax denominator.
"""

import sys

try:
    import concourse.bass as bass  # noqa: F401
except Exception:
    sys.path.insert(0, "/opt/trn_rl_repo")

import numpy as np

import concourse.bass as bass  # noqa: F401
import concourse.mybir as mybir
from concourse import bacc, bass_utils
from concourse.tile import TileContext

F32 = mybir.dt.float32
F32R = mybir.dt.float32r

D = 1024
S = 128
H = 16
DH = 64
C = 256          # attention window / block size
T = 4096
NCORES = 8
TOWN = T // NCORES        # 512 own tokens per core
HALO = C                  # 256 halo tokens
TLOC = TOWN + HALO        # 768 local rows per core
EPS = 1e-5

# bias-pack column layout
BC_A = 0
BC_QKV = 1       # 24 cols
BC_GATE = 25     # 8
BC_DRIVE = 33    # 1
BC_O = 34        # 8
BC_B1 = 42       # 32
BC_B2 = 74       # 8
NBC = 82

T_TILES = [(0, 512), (512, 256)]
OWN0 = HALO


def _r(ap):
    """Identity: matmul operands are natively float32r-typed."""
    return ap


def build_program(reps=1):
    nc = bacc.Bacc("TRN2", target_bir_lowering=False, debug=False)
    dt = F32
    d_xT = nc.dram_tensor("xT", [D, TLOC], F32R, kind="ExternalInput").ap()
    d_mask0 = nc.dram_tensor("mask0", [128, 4 * C], dt, kind="ExternalInput").ap()
    d_mask1 = nc.dram_tensor("mask1", [128, 4 * C], dt, kind="ExternalInput").ap()
    d_bias = nc.dram_tensor("biaspack", [128, NBC], dt, kind="ExternalInput").ap()
    d_vbias = nc.dram_tensor("vbias", [1, D], F32R, kind="ExternalInput").ap()
    d_wqkv = nc.dram_tensor("wqkv", [D, 3 * D], F32R, kind="ExternalInput").ap()
    d_wgate = nc.dram_tensor("wgate", [D, D], F32R, kind="ExternalInput").ap()
    d_wdrive = nc.dram_tensor("wdrive", [D, S], F32R, kind="ExternalInput").ap()
    d_wo = nc.dram_tensor("wo", [D, D], F32R, kind="ExternalInput").ap()
    d_cw = nc.dram_tensor("cw", [S, D], F32R, kind="ExternalInput").ap()
    d_w1 = nc.dram_tensor("w1", [D, 4 * D], F32R, kind="ExternalInput").ap()
    d_w2 = nc.dram_tensor("w2", [4 * D, D], F32R, kind="ExternalInput").ap()
    d_out = nc.dram_tensor("outT", [D, TOWN], dt, kind="ExternalOutput").ap()

    AF = mybir.ActivationFunctionType
    OP = mybir.AluOpType

    def persist(pool, shape, tag, dtype=F32):
        return pool.tile(shape, dtype, tag=tag, name=tag, bufs=1)

    def _tsub(o, a, b):
        nc.vector.tensor_sub(o, a, b)

    with TileContext(nc) as tc:
        for _rep in range(reps):
            with tc.tile_pool(name="const", bufs=1) as const, \
                 tc.tile_pool(name="xmid", bufs=1) as xm_pool:
                biasp = persist(const, [128, NBC], "biasp")
                nc.sync.dma_start(biasp[:], d_bias[:])
                mask0 = persist(const, [128, 4 * C], "mask0")
                nc.sync.dma_start(mask0[:], d_mask0[:])
                mask1 = persist(const, [128, 4 * C], "mask1")
                nc.sync.dma_start(mask1[:], d_mask1[:])
                ones_f = persist(const, [128, 128], "ones_f")
                nc.vector.memset(ones_f[:], 1.0 / D)
                ones_ln = persist(const, [128, 128], "ones_ln", F32R)
                nc.scalar.activation(ones_ln[:], ones_f[:], AF.Copy, bias=0.0)
                vbrow = persist(const, [1, D], "vbrow", F32R)
                nc.sync.dma_start(vbrow[:], d_vbias[:])
                vbias = persist(const, [128, D], "vbias")
                a_full = persist(const, [128, TLOC], "a_full")
                nc.vector.memset(a_full[:], 1.0)
                nc.vector.tensor_scalar_mul(a_full[:], a_full[:],
                                            biasp[:, BC_A:BC_A + 1])
                eps_col = persist(const, [128, 1], "eps_col")
                nc.vector.memset(eps_col[:], EPS)
                onesr_f = persist(const, [1, 128], "onesr_f")
                nc.vector.memset(onesr_f[:], 1.0)
                ones_row = persist(const, [1, 128], "ones_row", F32R)
                nc.scalar.activation(ones_row[:], onesr_f[:], AF.Copy, bias=0.0)

                def bias_col(idx):
                    return biasp[:, idx:idx + 1]

                def layernorm(xs, tfree, stats, spsum, out_pool, tagp):
                    mu = persist(stats, [128, tfree], f"mu{tagp}")
                    var = persist(stats, [128, tfree], f"var{tagp}")
                    rstd = persist(stats, [128, tfree], f"rstd{tagp}")
                    ttl = [(t0, tw) for (t0, tw) in T_TILES if t0 < tfree]
                    for t0, tw in ttl:
                        ps_mu = spsum.tile([128, 512], F32, tag="ln_mu")
                        ps_sq = spsum.tile([128, 512], F32, tag="ln_sq")
                        for c in range(8):
                            nc.tensor.matmul(ps_mu[:, :tw], _r(ones_ln[:]),
                                             _r(xs[c][:, t0:t0 + tw]),
                                             start=(c == 0), stop=(c == 7))
                        nc.vector.tensor_copy(mu[:, t0:t0 + tw], ps_mu[:, :tw])
                        for c in range(8):
                            sq = stats.tile([128, 512], F32R, tag="sq", bufs=3)
                            nc.scalar.activation(sq[:, :tw],
                                                 xs[c][:, t0:t0 + tw],
                                                 AF.Square)
                            nc.tensor.matmul(ps_sq[:, :tw], _r(ones_ln[:]),
                                             _r(sq[:, :tw]),
                                             start=(c == 0), stop=(c == 7))
                        nc.vector.tensor_mul(var[:, t0:t0 + tw], mu[:, t0:t0 + tw],
                                             mu[:, t0:t0 + tw])
                        _tsub(var[:, t0:t0 + tw], ps_sq[:, :tw], var[:, t0:t0 + tw])
                        nc.scalar.activation(rstd[:, t0:t0 + tw],
                                             var[:, t0:t0 + tw], AF.Sqrt,
                                             bias=eps_col[:])
                        nc.vector.reciprocal(rstd[:, t0:t0 + tw],
                                             rstd[:, t0:t0 + tw])
                    outs = []
                    for c in range(8):
                        o = persist(out_pool, [128, tfree], f"{tagp}{c}", F32R)
                        _tsub(o[:], xs[c][:], mu[:])
                        nc.vector.tensor_mul(o[:], o[:], rstd[:])
                        outs.append(o)
                    return outs

                # ====== phase group A: LN1, projections, attention, fusion =====
                with tc.tile_pool(name="act1", bufs=1) as act1:
                    gs = [persist(act1, [128, TOWN], f"g{c}") for c in range(8)]
                    attns = [persist(act1, [128, TOWN], f"at{c}", F32R) for c in range(8)]
                    driveT = persist(act1, [128, TLOC], "driveT")
                    states = persist(act1, [128, TLOC], "states", F32R)

                    with tc.tile_pool(name="act0", bufs=1) as act0:
                        kts = [persist(act0, [128, TLOC], f"k{c}", F32R) for c in range(8)]
                        vts = [persist(act0, [128, 16 * 65], f"v{c}", F32R)
                               for c in range(6)]
                        qts = [persist(act0, [128, TOWN], f"q{c}", F32R) for c in range(8)]
                        vones = persist(act0, [128, 16], "vones")
                        nc.vector.memset(vones[:], 1.0)
                        for tt in range(6):
                            vr = vts[tt][:].rearrange("p (h e) -> p h e", e=65)
                            nc.scalar.activation(
                                vr[:, :, 64:65],
                                vones[:].rearrange("p (a b) -> p a b", b=1),
                                AF.Copy, bias=0.0)

                        xn_pool_outer = act0
                        with tc.tile_pool(name="xT", bufs=1) as xtp, \
                             tc.tile_pool(name="lnst", bufs=1) as lnst, \
                             tc.tile_pool(name="lnpsum", bufs=2,
                                          space="PSUM") as lnp:
                            xts = [persist(xtp, [128, TLOC], f"x{c}", F32R)
                                   for c in range(8)]
                            for c in range(8):
                                nc.sync.dma_start(
                                    xts[c][:], d_xT[c * 128:(c + 1) * 128, :])
                            xns = layernorm(xts, TLOC, lnst, lnp,
                                            xn_pool_outer, "xn")

                        # ---- projections -----------------------------------
                        with tc.tile_pool(name="wlin", bufs=12) as wp, \
                             tc.tile_pool(name="linpsum", bufs=6,
                                          space="PSUM") as psum:

                            for half2 in range(2):
                                psb = psum.tile([128, 512], F32, tag="lin")
                                nc.tensor.matmul(
                                    psb[:], ones_row[:],
                                    vbrow[:, half2 * 512:(half2 + 1) * 512],
                                    start=True, stop=True)
                                nc.vector.tensor_copy(
                                    vbias[:, half2 * 512:(half2 + 1) * 512],
                                    psb[:])

                            def load_w(dram, kc, m0, mw):
                                w = wp.tile([128, 512], F32R, tag="w")
                                nc.sync.dma_start(
                                    w[:, :mw],
                                    dram[kc * 128:(kc + 1) * 128, m0:m0 + mw])
                                return w

                            # K chunks (qkv cols 1024..2048), all 768 rows
                            for mg in range(2):
                                ws = [load_w(d_wqkv, kc, D + mg * 512, 512)
                                      for kc in range(8)]
                                for j in range(4):
                                    mc = mg * 4 + j
                                    for t0, tw in T_TILES:
                                        ps = psum.tile([128, 512], F32, tag="lin")
                                        for kc in range(8):
                                            nc.tensor.matmul(
                                                ps[:, :tw],
                                                _r(ws[kc][:, j * 128:(j + 1) * 128]),
                                                _r(xns[kc][:, t0:t0 + tw]),
                                                start=(kc == 0), stop=(kc == 7))
                                        nc.vector.tensor_scalar(
                                            kts[mc][:, t0:t0 + tw], ps[:, :tw],
                                            bias_col(BC_QKV + 8 + mc), None, OP.add)
                            # V chunks (qkv cols 2048..3072), token-major
                            for vc in range(2):
                                ws = [load_w(d_wqkv, kc, 2 * D + vc * 512, 512)
                                      for kc in range(8)]
                                for tt in range(6):
                                    t0 = tt * 128
                                    ps = psum.tile([128, 512], F32, tag="lin")
                                    for kc in range(8):
                                        nc.tensor.matmul(
                                            ps[:], _r(xns[kc][:, t0:t0 + 128]),
                                            _r(ws[kc][:]),
                                            start=(kc == 0), stop=(kc == 7))
                                    vr = vts[tt][:].rearrange(
                                        "p (h e) -> p h e", e=65)
                                    dst = vr[:, vc * 8:(vc + 1) * 8, 0:64]
                                    nc.vector.tensor_copy(
                                        dst,
                                        ps[:].rearrange("p (h e) -> p h e",
                                                        e=64))
                                    nc.vector.tensor_add(
                                        dst, dst,
                                        vbias[:, vc * 512:(vc + 1) * 512].rearrange(
                                            "p (h e) -> p h e", e=64))
                            # Q chunks (qkv cols 0..1024), own rows only
                            for mg in range(2):
                                ws = [load_w(d_wqkv, kc, mg * 512, 512)
                                      for kc in range(8)]
                                for j in range(4):
                                    mc = mg * 4 + j
                                    ps = psum.tile([128, 512], F32, tag="lin")
                                    for kc in range(8):
                                        nc.tensor.matmul(
                                            ps[:],
                                            _r(ws[kc][:, j * 128:(j + 1) * 128]),
                                            _r(xns[kc][:, OWN0:OWN0 + TOWN]),
                                            start=(kc == 0), stop=(kc == 7))
                                    nc.vector.tensor_scalar(
                                        qts[mc][:], ps[:], bias_col(BC_QKV + mc),
                                        None, OP.add)
                            # gate (sigmoid), own rows only
                            for mg in range(2):
                                ws = [load_w(d_wgate, kc, mg * 512, 512)
                                      for kc in range(8)]
                                for j in range(4):
                                    mc = mg * 4 + j
                                    ps = psum.tile([128, 512], F32, tag="lin")
                                    for kc in range(8):
                                        nc.tensor.matmul(
                                            ps[:],
                                            _r(ws[kc][:, j * 128:(j + 1) * 128]),
                                            _r(xns[kc][:, OWN0:OWN0 + TOWN]),
                                            start=(kc == 0), stop=(kc == 7))
                                    nc.scalar.activation(gs[mc][:], ps[:],
                                                         AF.Sigmoid,
                                                         bias=bias_col(BC_GATE + mc))
                            # drive, all 768 rows
                            ws = [load_w(d_wdrive, kc, 0, 128) for kc in range(8)]
                            for t0, tw in T_TILES:
                                ps = psum.tile([128, 512], F32, tag="lin")
                                for kc in range(8):
                                    nc.tensor.matmul(
                                        ps[:, :tw], _r(ws[kc][:, :128]),
                                        _r(xns[kc][:, t0:t0 + tw]),
                                        start=(kc == 0), stop=(kc == 7))
                                nc.vector.tensor_scalar(
                                    driveT[:, t0:t0 + tw], ps[:, :tw],
                                    bias_col(BC_DRIVE), None, OP.add)

                        # ---- windowed attention ----------------------------
                        with tc.tile_pool(name="apsum", bufs=3,
                                          space="PSUM") as apsum, \
                             tc.tile_pool(name="ptp", bufs=12) as ptp, \
                             tc.tile_pool(name="rp", bufs=3) as rp:
                            for b in range(2):
                                mask = mask0 if b == 0 else mask1
                                for h in range(H):
                                    cch = h // 2
                                    half = (h % 2) * 64
                                    pts = []
                                    for kc in range(4):
                                        st = apsum.tile([128, C], F32, tag="st")
                                        k0 = C * b + 128 * kc
                                        nc.tensor.matmul(
                                            st[:],
                                            _r(kts[cch][half:half + 64,
                                                        k0:k0 + 128]),
                                            _r(qts[cch][half:half + 64,
                                                        C * b:C * (b + 1)]),
                                            start=True, stop=True)
                                        pt = ptp.tile([128, C], F32R, tag="pt")
                                        nc.scalar.activation(
                                            pt[:], st[:], AF.Exp,
                                            scale=float(1.0 / np.sqrt(DH)))
                                        nc.vector.tensor_mul(
                                            pt[:], pt[:],
                                            mask[:, kc * C:(kc + 1) * C])
                                        pts.append(pt)
                                    po = apsum.tile([65, C], F32, tag="po",
                                                    bufs=3)
                                    for kc in range(4):
                                        nc.tensor.matmul(
                                            po[:],
                                            _r(vts[2 * b + kc][:,
                                               h * 65:(h + 1) * 65]),
                                            _r(pts[kc][:]),
                                            start=(kc == 0), stop=(kc == 3))
                                    rrow = rp.tile([1, C], F32R, tag="rr")
                                    with nc.allow_low_precision(
                                            reason="f32r rounding of softmax "
                                            "denominators is benign"):
                                        nc.vector.reciprocal(rrow[:],
                                                             po[64:65, :])
                                    rb = apsum.tile([64, C], F32, tag="rb",
                                                    bufs=2)
                                    nc.tensor.matmul(rb[:],
                                                     ones_row[:, :64],
                                                     rrow[:],
                                                     start=True, stop=True)
                                    rs = rp.tile([64, C], F32, tag="r64")
                                    nc.vector.tensor_copy(rs[:], rb[:])
                                    nc.vector.tensor_mul(
                                        attns[cch][half:half + 64,
                                                   C * b:C * (b + 1)],
                                        po[0:64, :], rs[:])
                    # act0 closed: xn/kt/v/qt freed

                    # ---- SSM scan + projections + fusion -------------------
                    nc.vector.tensor_tensor_scan(states[:], a_full[:], driveT[:],
                                                 0.0, OP.mult, OP.add)
                    with tc.tile_pool(name="fus", bufs=1) as fus, \
                         tc.tile_pool(name="wfus", bufs=10) as wf, \
                         tc.tile_pool(name="spsum", bufs=4, space="PSUM") as sp:
                        ys = [persist(fus, [128, TOWN], f"y{c}") for c in range(8)]
                        xos = [persist(fus, [128, TOWN], f"xo{c}", F32R) for c in range(8)]
                        for c in range(8):
                            nc.sync.dma_start(
                                xos[c][:],
                                d_xT[c * 128:(c + 1) * 128, OWN0:OWN0 + TOWN])
                        wcs = []
                        for mg in range(2):
                            w = wf.tile([128, 512], F32R, tag="w")
                            nc.sync.dma_start(w[:], d_cw[:, mg * 512:(mg + 1) * 512])
                            wcs.append(w)
                        for mc in range(8):
                            ps = sp.tile([128, 512], F32, tag="s")
                            nc.tensor.matmul(
                                ps[:],
                                _r(wcs[mc // 4][:, (mc % 4) * 128:(mc % 4 + 1) * 128]),
                                _r(states[:, OWN0:OWN0 + TOWN]),
                                start=True, stop=True)
                            nc.vector.tensor_copy(ys[mc][:], ps[:])
                        xms = [persist(xm_pool, [128, TOWN], f"xm{c}", F32R)
                               for c in range(8)]
                        for mg in range(2):
                            wos = [load_w_pool(wf, nc, d_wo, kc, mg * 512, 512)
                                   for kc in range(8)]
                            for j in range(4):
                                mc = mg * 4 + j
                                ps = sp.tile([128, 512], F32, tag="s")
                                for kc in range(8):
                                    nc.tensor.matmul(
                                        ps[:],
                                        _r(wos[kc][:, j * 128:(j + 1) * 128]),
                                        _r(attns[kc][:]),
                                        start=(kc == 0), stop=(kc == 7))
                                xm = xms[mc]
                                nc.vector.scalar_tensor_tensor(
                                    xm[:], ps[:], bias_col(BC_O + mc), ys[mc][:],
                                    op0=OP.add, op1=OP.subtract)
                                nc.vector.tensor_mul(xm[:], xm[:], gs[mc][:])
                                nc.vector.tensor_add(xm[:], xm[:], ys[mc][:])
                                nc.vector.tensor_add(xm[:], xm[:], xos[mc][:])
                # act1 closed: g/attn/drive/states freed

                # ====== phase group B: LN2 + MLP ===============================
                with tc.tile_pool(name="xn2p", bufs=1) as xn2p:
                    with tc.tile_pool(name="lnst2", bufs=1) as lnst2, \
                         tc.tile_pool(name="ln2psum", bufs=2, space="PSUM") as lnp2:
                        xn2s = layernorm(xms, TOWN, lnst2, lnp2, xn2p, "h")
                    with tc.tile_pool(name="hTp", bufs=1) as hTp, \
                         tc.tile_pool(name="wmlp", bufs=12) as wm:
                        hts = [persist(hTp, [128, TOWN], f"ht{c}", F32R)
                               for c in range(32)]
                        with tc.tile_pool(name="m1psum", bufs=6,
                                          space="PSUM") as mp1:
                            for mg in range(8):
                                ws = [load_w_pool(wm, nc, d_w1, kc, mg * 512, 512)
                                      for kc in range(8)]
                                for j in range(4):
                                    mc = mg * 4 + j
                                    ps = mp1.tile([128, 512], F32, tag="m")
                                    for kc in range(8):
                                        nc.tensor.matmul(
                                            ps[:],
                                            _r(ws[kc][:, j * 128:(j + 1) * 128]),
                                            _r(xn2s[kc][:]),
                                            start=(kc == 0), stop=(kc == 7))
                                    nc.scalar.activation(
                                        hts[mc][:], ps[:], AF.Gelu,
                                        bias=bias_col(BC_B1 + mc))
                        with tc.tile_pool(name="m2psum", bufs=1,
                                          space="PSUM") as mp2, \
                             tc.tile_pool(name="outp", bufs=3) as outp:
                            pss = [mp2.tile([128, 512], F32, tag=f"o{mc}",
                                            name=f"o{mc}", bufs=1)
                                   for mc in range(8)]
                            for kc in range(32):
                                w2r = wm.tile([128, 1024], F32R, tag="w2", bufs=3)
                                nc.sync.dma_start(
                                    w2r[:], d_w2[kc * 128:(kc + 1) * 128, :])
                                for mc in range(8):
                                    nc.tensor.matmul(
                                        pss[mc][:],
                                        _r(w2r[:, mc * 128:(mc + 1) * 128]),
                                        _r(hts[kc][:]),
                                        start=(kc == 0), stop=(kc == 31))
                            for mc in range(8):
                                oc = outp.tile([128, TOWN], F32, tag="oc")
                                nc.vector.scalar_tensor_tensor(
                                    oc[:], pss[mc][:], bias_col(BC_B2 + mc),
                                    xms[mc][:], op0=OP.add, op1=OP.add)
                                nc.sync.dma_start(
                                    d_out[mc * 128:(mc + 1) * 128, :], oc[:])

    nc.compile()
    return nc


def load_w_pool(pool, nc, dram, kc, m0, mw):
    w = pool.tile([128, 512], F32R, tag="w")
    nc.sync.dma_start(w[:, :mw], dram[kc * 128:(kc + 1) * 128, m0:m0 + mw])
    return w


def _make_masks():
    qi = np.arange(C)[:, None]
    kk = np.arange(2 * C)[None, :]
    band = (kk > qi) & (kk <= qi + C)
    first = band & (kk >= C)

    def pack(m):                       # [C, 2C] -> [128, 4*C] k-chunk-major
        mt = m.T.astype(np.float32)    # [2C, C]
        return np.ascontiguousarray(
            mt.reshape(4, 128, C).transpose(1, 0, 2).reshape(128, 4 * C))

    return pack(first), pack(band)


def _prep_inputs(x, ln1_g, ln1_b, ln2_g, ln2_b, W_qkv, W_O, b_O, W_ug, b_ug,
                 B_w, A, C_w, mlp_W1, mlp_b1, mlp_W2, mlp_b2):
    f = np.float32
    g1 = np.asarray(ln1_g, f)
    b1 = np.asarray(ln1_b, f)
    W_qkv = np.asarray(W_qkv, f)
    W_qkv_e = g1[:, None] * W_qkv
    b_qkv_e = b1 @ W_qkv
    W_ug = np.asarray(W_ug, f)
    B_w = np.asarray(B_w, f)
    b_ug = np.asarray(b_ug, f)
    W_drive_raw = B_w + W_ug[:, :S]
    W_drive_e = g1[:, None] * W_drive_raw
    b_drive_e = b1 @ W_drive_raw + b_ug[:S]
    W_gate_e = g1[:, None] * W_ug[:, S:]
    b_gate_e = b1 @ W_ug[:, S:] + b_ug[S:]
    g2 = np.asarray(ln2_g, f)
    b2l = np.asarray(ln2_b, f)
    mlp_W1 = np.asarray(mlp_W1, f)
    W1_e = g2[:, None] * mlp_W1
    b1_e = b2l @ mlp_W1 + np.asarray(mlp_b1, f)

    biaspack = np.zeros((128, NBC), f)
    biaspack[:, BC_A] = np.asarray(A, f)
    biaspack[:, BC_QKV:BC_QKV + 24] = b_qkv_e.reshape(24, 128).T
    biaspack[:, BC_GATE:BC_GATE + 8] = b_gate_e.reshape(8, 128).T
    biaspack[:, BC_DRIVE] = b_drive_e
    biaspack[:, BC_O:BC_O + 8] = np.asarray(b_O, f).reshape(8, 128).T
    biaspack[:, BC_B1:BC_B1 + 32] = b1_e.reshape(32, 128).T
    biaspack[:, BC_B2:BC_B2 + 8] = np.asarray(mlp_b2, f).reshape(8, 128).T
    vbias = np.ascontiguousarray(b_qkv_e[2 * D:].reshape(1, D))

    m_first, m_band = _make_masks()
    xTfull = np.ascontiguousarray(np.asarray(x, f)[0].T)

    shared = {
        "biaspack": biaspack, "vbias": vbias,
        "wqkv": np.ascontiguousarray(W_qkv_e),
        "wgate": np.ascontiguousarray(W_gate_e),
        "wdrive": np.ascontiguousarray(W_drive_e),
        "wo": np.ascontiguousarray(np.asarray(W_O, f)),
        "cw": np.ascontiguousarray(np.asarray(C_w, f)),
        "w1": np.ascontiguousarray(W1_e),
        "w2": np.ascontiguousarray(np.asarray(mlp_W2, f)),
        "mask1": m_band,
    }
    in_maps = []
    for i in range(NCORES):
        t0 = i * TOWN
        xT = np.zeros((D, TLOC), f)
        lo = max(0, t0 - HALO)
        xT[:, HALO - (t0 - lo):HALO] = xTfull[:, lo:t0]
        xT[:, HALO:] = xTfull[:, t0:t0 + TOWN]
        m0 = m_first if i == 0 else m_band
        in_maps.append({**shared, "xT": np.ascontiguousarray(xT), "mask0": m0})
    return in_maps


_CACHED_NC = None


def get_nc():
    global _CACHED_NC
    if _CACHED_NC is None:
        _CACHED_NC = build_program()
    return _CACHED_NC


def kernel(**inputs):
    nc = get_nc()
    in_maps = _prep_inputs(**inputs)
    res = bass_utils.run_bass_kernel_spmd(nc, in_maps,
                                          core_ids=list(range(NCORES)))
    out = np.empty((1, T, D), np.float32)
    for i in range(NCORES):
        out[0, i * TOWN:(i + 1) * TOWN, :] = res.results[i]["outT"].T
    return out



# revision 34
# speedup vs baseline: 32.4106x; 32.4106x over previous
"""DPA+SSM block kernel for 8 Trainium2 NeuronCores.

Sharding: data-parallel over the sequence (T=4096 -> 8 x 512 own tokens);
each core also receives a 256-token halo of the raw input before its own
range.  The attention window is 256, so the halo covers every key a core
needs; the SSM recurrence decay |A| < 0.1 makes state influence from before
the halo underflow fp32 entirely, so a zero-initialized scan warm-started
over the halo is exact.  No cross-core communication.

v2: bf16 weights + bf16 matmul activations (fp32 PSUM accumulation),
batched softmax with a single-op DVE approx reciprocal, software-pipelined
attention heads, early SSM scan, W1 half-prefetch during attention.
LayerNorm keeps fp32 statistics via an all-ones stationary matmul; the
scan is one exact DVE tensor_tensor_scan in fp32.
"""

import sys

try:
    import concourse.bass as bass  # noqa: F401
except Exception:
    sys.path.insert(0, "/opt/trn_rl_repo")

import numpy as np
import ml_dtypes

import concourse.bass as bass  # noqa: F401
import concourse.mybir as mybir
from concourse import bacc, bass_utils
from concourse.tile import TileContext

F32 = mybir.dt.float32
F32R = mybir.dt.float32r
BF16 = mybir.dt.bfloat16
NPBF = ml_dtypes.bfloat16

D = 1024
S = 128
H = 16
DH = 64
C = 256          # attention window / block size
T = 4096
NCORES = 8
TOWN = T // NCORES        # 512 own tokens per core
HALO = C                  # 256 halo tokens
TLOC = TOWN + HALO        # 768 local rows per core
EPS = 1e-5

# bias-pack column layout (fp32 [128, NBC])
BC_A = 0
BC_QKV = 1       # 24 cols (Q 0-7, K 8-15, V 16-23; V via vbias instead)
BC_GATE = 25     # 8
BC_DRIVE = 33    # 1
BC_O = 34        # 8
BC_B1 = 42       # 32
BC_B2 = 74       # 8
NBC = 82

T_TILES = [(0, 512), (512, 256)]
OWN0 = HALO


def build_program(reps=1, taps=False):
    nc = bacc.Bacc("TRN2", target_bir_lowering=False, debug=False)
    d_tap = {}
    if taps:
        for nm, shape, dt in [
                ("dbg_xn", [128, TLOC], BF16), ("dbg_k", [128, TLOC], BF16),
                ("dbg_q", [128, TOWN], BF16), ("dbg_v", [128, 16 * 128], BF16),
                ("dbg_s", [128, TLOC], F32R), ("dbg_y", [128, TOWN], BF16),
                ("dbg_g", [128, TOWN], BF16), ("dbg_at", [128, TOWN], BF16),
                ("dbg_xm", [128, TOWN], F32R), ("dbg_xn2", [128, TOWN], BF16),
                ("dbg_h", [128, TOWN], BF16),
                ("dbg_pt", [128, 4 * C], BF16), ("dbg_den", [1, C], F32),
                ("dbg_rr", [1, C], F32), ("dbg_rb", [64, C], F32),
                ("dbg_po", [128, C], F32)]:
            d_tap[nm] = nc.dram_tensor(nm, shape, dt,
                                       kind="ExternalOutput").ap()

    def tap(nm, tile):
        if taps:
            nc.sync.dma_start(d_tap[nm][:], tile[:])

    d_xT = nc.dram_tensor("xT", [D, TLOC], BF16, kind="ExternalInput").ap()
    d_xown = nc.dram_tensor("xown", [D, TOWN], F32, kind="ExternalInput").ap()
    d_mask0 = nc.dram_tensor("mask0", [128, 4 * C], BF16, kind="ExternalInput").ap()
    d_mask1 = nc.dram_tensor("mask1", [128, 4 * C], BF16, kind="ExternalInput").ap()
    d_bias = nc.dram_tensor("biaspack", [128, NBC], F32, kind="ExternalInput").ap()
    d_vbias = nc.dram_tensor("vbias", [128, D], BF16, kind="ExternalInput").ap()
    d_wqkv = nc.dram_tensor("wqkv", [D, 3 * D], BF16, kind="ExternalInput").ap()
    d_wgate = nc.dram_tensor("wgate", [D, D], BF16, kind="ExternalInput").ap()
    d_wdrive = nc.dram_tensor("wdrive", [D, S], BF16, kind="ExternalInput").ap()
    d_wo = nc.dram_tensor("wo", [D, D], BF16, kind="ExternalInput").ap()
    d_cw = nc.dram_tensor("cw", [S, D], F32R, kind="ExternalInput").ap()
    d_w1 = nc.dram_tensor("w1", [D, 4 * D], BF16, kind="ExternalInput").ap()
    d_w2 = nc.dram_tensor("w2", [4 * D, D], BF16, kind="ExternalInput").ap()
    d_out = nc.dram_tensor("outT", [D, TOWN], F32, kind="ExternalOutput").ap()

    AF = mybir.ActivationFunctionType
    OP = mybir.AluOpType

    def persist(pool, shape, tag, dtype=F32):
        return pool.tile(shape, dtype, tag=tag, name=tag, bufs=1)

    with TileContext(nc) as tc:
        for _rep in range(reps):
            with tc.tile_pool(name="const", bufs=1) as const, \
                 tc.tile_pool(name="xmid", bufs=1) as xm_pool, \
                 tc.tile_pool(name="w1p", bufs=1) as w1p:
                biasp = persist(const, [128, NBC], "biasp")
                nc.sync.dma_start(biasp[:], d_bias[:])
                mask0 = persist(const, [128, 4 * C], "mask0", BF16)
                nc.sync.dma_start(mask0[:], d_mask0[:])
                mask1 = persist(const, [128, 4 * C], "mask1", BF16)
                nc.sync.dma_start(mask1[:], d_mask1[:])
                vbias = persist(const, [128, D], "vbias", BF16)
                nc.sync.dma_start(vbias[:], d_vbias[:])
                # stationary all-ones (value 1/D) for LN partition reduction
                ones_f = persist(const, [128, 128], "ones_f")
                nc.vector.memset(ones_f[:], 1.0 / D)
                ones_lnb = persist(const, [128, 128], "ones_lnb", BF16)
                nc.scalar.activation(ones_lnb[:], ones_f[:], AF.Copy, bias=0.0)
                ones_lnr = persist(const, [128, 128], "ones_lnr", F32R)
                nc.scalar.activation(ones_lnr[:], ones_f[:], AF.Copy, bias=0.0)
                a_full = persist(const, [128, TLOC], "a_full")
                nc.vector.memset(a_full[:], 1.0)
                nc.vector.tensor_scalar_mul(a_full[:], a_full[:],
                                            biasp[:, BC_A:BC_A + 1])
                eps_col = persist(const, [128, 1], "eps_col")
                nc.vector.memset(eps_col[:], EPS)
                onesr_f = persist(const, [1, 128], "onesr_f")
                nc.vector.memset(onesr_f[:], 1.0)
                ones_row_r = persist(const, [1, 128], "ones_row_r", F32R)
                nc.scalar.activation(ones_row_r[:], onesr_f[:], AF.Copy,
                                     bias=0.0)

                def bias_col(idx):
                    return biasp[:, idx:idx + 1]

                def layernorm(xs, tfree, stats, spsum, out_pool, tagp, ones_st,
                              sq_dt):
                    """fp32 stats from PSUM; bf16 mu + approx-recip rstd."""
                    mu = persist(stats, [128, tfree], f"mu{tagp}", BF16)
                    rstd = persist(stats, [128, tfree], f"rstd{tagp}")
                    var = persist(stats, [128, tfree], f"var{tagp}")
                    musq = persist(stats, [128, tfree], f"musq{tagp}")
                    ttl = [(t0, tw) for (t0, tw) in T_TILES if t0 < tfree]
                    for t0, tw in ttl:
                        ps_mu = spsum.tile([128, 512], F32, tag="ln_mu")
                        ps_sq = spsum.tile([128, 512], F32, tag="ln_sq")
                        for c in range(8):
                            nc.tensor.matmul(ps_mu[:, :tw], ones_st[:],
                                             xs[c][:, t0:t0 + tw],
                                             start=(c == 0), stop=(c == 7))
                        nc.vector.tensor_copy(mu[:, t0:t0 + tw], ps_mu[:, :tw])
                        for c in range(8):
                            sq = stats.tile([128, 512], sq_dt, tag="sq", bufs=3)
                            nc.scalar.activation(sq[:, :tw],
                                                 xs[c][:, t0:t0 + tw],
                                                 AF.Square)
                            nc.tensor.matmul(ps_sq[:, :tw], ones_st[:],
                                             sq[:, :tw],
                                             start=(c == 0), stop=(c == 7))
                        nc.vector.tensor_mul(musq[:, t0:t0 + tw],
                                             mu[:, t0:t0 + tw],
                                             mu[:, t0:t0 + tw])
                        nc.vector.tensor_sub(var[:, t0:t0 + tw],
                                             ps_sq[:, :tw],
                                             musq[:, t0:t0 + tw])
                        nc.scalar.activation(musq[:, t0:t0 + tw],
                                             var[:, t0:t0 + tw], AF.Sqrt,
                                             bias=eps_col[:])
                        nc.vector.reciprocal_approx_fast(
                            rstd[:, t0:t0 + tw], musq[:, t0:t0 + tw])
                    outs = []
                    for c in range(8):
                        o = persist(out_pool, [128, tfree], f"{tagp}{c}", BF16)
                        nc.vector.tensor_sub(o[:], xs[c][:], mu[:])
                        nc.vector.tensor_mul(o[:], o[:], rstd[:])
                        outs.append(o)
                    return outs

                # ====== phase group A: LN1, scan, projections, attention ======
                with tc.tile_pool(name="act1", bufs=1) as act1:
                    gs = [persist(act1, [128, TOWN], f"g{c}", BF16)
                          for c in range(8)]
                    attns = [persist(act1, [128, TOWN], f"at{c}", BF16)
                             for c in range(8)]
                    driveT = persist(act1, [128, TLOC], "driveT")
                    states = persist(act1, [128, TLOC], "states", F32R)
                    ys = [persist(act1, [128, TOWN], f"y{c}", BF16)
                          for c in range(8)]
                    yxos = [persist(act1, [128, TOWN], f"yx{c}")
                            for c in range(8)]

                    with tc.tile_pool(name="act0", bufs=1) as act0:
                        kts = [persist(act0, [128, TLOC], f"k{c}", BF16)
                               for c in range(8)]
                        vts = [persist(act0, [128, 16 * 128], f"v{c}", BF16)
                               for c in range(6)]
                        qts = [persist(act0, [128, TOWN], f"q{c}", BF16)
                               for c in range(8)]
                        vones = persist(act0, [128, 16], "vones", BF16)
                        nc.vector.memset(vones[:], 1.0)
                        for tt in range(6):
                            nc.vector.memset(vts[tt][:], 0.0)
                        for tt in range(6):
                            # ones column FIRST: den lands in po row 0 (the
                            # custom-DVE reciprocal ignores partition offsets)
                            vr = vts[tt][:].rearrange("p (h e) -> p h e",
                                                      e=128)
                            nc.scalar.activation(
                                vr[:, :, 0:1],
                                vones[:].rearrange("p (a b) -> p a b", b=1),
                                AF.Copy, bias=0.0)

                        with tc.tile_pool(name="wlin", bufs=12) as wp, \
                             tc.tile_pool(name="linpsum", bufs=2,
                                          space="PSUM") as psum:

                            def load_w(dram, kc, m0, mw):
                                w = wp.tile([128, 512], BF16, tag="w")
                                nc.sync.dma_start(
                                    w[:, :mw],
                                    dram[kc * 128:(kc + 1) * 128, m0:m0 + mw])
                                return w

                            with tc.tile_pool(name="xT", bufs=1) as xtp, \
                                 tc.tile_pool(name="lnst", bufs=1) as lnst, \
                                 tc.tile_pool(name="lnpsum", bufs=2,
                                              space="PSUM") as lnp:
                                xts = [persist(xtp, [128, TLOC], f"x{c}", BF16)
                                       for c in range(8)]
                                for c in range(8):
                                    nc.sync.dma_start(
                                        xts[c][:],
                                        d_xT[c * 128:(c + 1) * 128, :])
                                xns = layernorm(xts, TLOC, lnst, lnp,
                                                act0, "xn", ones_lnb, BF16)
                                tap("dbg_xn", xns[0])

                            # ---- drive -> scan -> y_ssm (global path) ------
                            ws = [load_w(d_wdrive, kc, 0, 128)
                                  for kc in range(8)]
                            for t0, tw in T_TILES:
                                ps = psum.tile([128, 512], F32, tag="lin")
                                for kc in range(8):
                                    nc.tensor.matmul(
                                        ps[:, :tw], ws[kc][:, :128],
                                        xns[kc][:, t0:t0 + tw],
                                        start=(kc == 0), stop=(kc == 7))
                                nc.vector.tensor_scalar(
                                    driveT[:, t0:t0 + tw], ps[:, :tw],
                                    bias_col(BC_DRIVE), None, OP.add)
                            nc.vector.tensor_tensor_scan(
                                states[:], a_full[:], driveT[:],
                                0.0, OP.mult, OP.add)
                            tap("dbg_s", states)

                            cwt = persist(wp, [128, D], "cw", F32R)
                            nc.sync.dma_start(cwt[:], d_cw[:])
                            with tc.tile_pool(name="xosp", bufs=1) as xosp:
                                xos = [persist(xosp, [128, TOWN], f"xo{c}")
                                       for c in range(8)]
                                for c in range(8):
                                    nc.gpsimd.dma_start(
                                        xos[c][:],
                                        d_xown[c * 128:(c + 1) * 128, :])
                                for mc in range(8):
                                    ps = psum.tile([128, 512], F32, tag="lin")
                                    nc.tensor.matmul(
                                        ps[:],
                                        cwt[:, mc * 128:(mc + 1) * 128],
                                        states[:, OWN0:OWN0 + TOWN],
                                        start=True, stop=True)
                                    nc.vector.tensor_copy(ys[mc][:], ps[:])
                                    nc.vector.tensor_add(
                                        yxos[mc][:], ps[:], xos[mc][:])
                            tap("dbg_y", ys[0])

                            # ---- K / V / Q / gate projections --------------
                            for mg in range(2):
                                ws = [load_w(d_wqkv, kc, D + mg * 512, 512)
                                      for kc in range(8)]
                                for j in range(4):
                                    mc = mg * 4 + j
                                    for t0, tw in T_TILES:
                                        ps = psum.tile([128, 512], F32,
                                                       tag="lin")
                                        for kc in range(8):
                                            nc.tensor.matmul(
                                                ps[:, :tw],
                                                ws[kc][:, j * 128:(j + 1) * 128],
                                                xns[kc][:, t0:t0 + tw],
                                                start=(kc == 0),
                                                stop=(kc == 7))
                                        nc.vector.tensor_scalar(
                                            kts[mc][:, t0:t0 + tw],
                                            ps[:, :tw],
                                            bias_col(BC_QKV + 8 + mc),
                                            None, OP.add)
                            tap("dbg_k", kts[0])
                            for vc in range(2):
                                ws = [load_w(d_wqkv, kc, 2 * D + vc * 512, 512)
                                      for kc in range(8)]
                                for tt in range(6):
                                    t0 = tt * 128
                                    ps = psum.tile([128, 512], F32, tag="lin")
                                    for kc in range(8):
                                        nc.tensor.matmul(
                                            ps[:],
                                            xns[kc][:, t0:t0 + 128],
                                            ws[kc][:],
                                            start=(kc == 0), stop=(kc == 7))
                                    vr = vts[tt][:].rearrange(
                                        "p (h e) -> p h e", e=128)
                                    dst = vr[:, vc * 8:(vc + 1) * 8, 64:128]
                                    nc.vector.scalar_tensor_tensor(
                                        dst, ps[:].rearrange(
                                            "p (h e) -> p h e", e=64),
                                        1.0,
                                        vbias[:, vc * 512:(vc + 1) * 512]
                                        .rearrange("p (h e) -> p h e", e=64),
                                        op0=OP.mult, op1=OP.add)
                            for mg in range(2):
                                ws = [load_w(d_wqkv, kc, mg * 512, 512)
                                      for kc in range(8)]
                                for j in range(4):
                                    mc = mg * 4 + j
                                    ps = psum.tile([128, 512], F32, tag="lin")
                                    for kc in range(8):
                                        nc.tensor.matmul(
                                            ps[:],
                                            ws[kc][:, j * 128:(j + 1) * 128],
                                            xns[kc][:, OWN0:OWN0 + TOWN],
                                            start=(kc == 0), stop=(kc == 7))
                                    nc.vector.tensor_scalar(
                                        qts[mc][:], ps[:],
                                        bias_col(BC_QKV + mc),
                                        None, OP.add)
                            for mg in range(2):
                                ws = [load_w(d_wgate, kc, mg * 512, 512)
                                      for kc in range(8)]
                                for j in range(4):
                                    mc = mg * 4 + j
                                    ps = psum.tile([128, 512], F32, tag="lin")
                                    for kc in range(8):
                                        nc.tensor.matmul(
                                            ps[:],
                                            ws[kc][:, j * 128:(j + 1) * 128],
                                            xns[kc][:, OWN0:OWN0 + TOWN],
                                            start=(kc == 0), stop=(kc == 7))
                                    nc.scalar.activation(
                                        gs[mc][:], ps[:], AF.Sigmoid,
                                        bias=bias_col(BC_GATE + mc))

                            tap("dbg_v", vts[0])
                            tap("dbg_q", qts[0])
                            tap("dbg_g", gs[0])
                            # ---- windowed attention (pipelined heads) ------
                            with tc.tile_pool(name="apsum", bufs=2,
                                              space="PSUM") as apsum, \
                                 tc.tile_pool(name="ptp", bufs=3) as ptp, \
                                 tc.tile_pool(name="rp", bufs=3) as rp:
                                # prefetch W1 first half while DMA is idle
                                w1ts = [persist(w1p, [128, 2 * D],
                                                f"w1_{kc}", BF16)
                                        for kc in range(8)]
                                for kc in range(8):
                                    nc.gpsimd.dma_start(
                                        w1ts[kc][:],
                                        d_w1[kc * 128:(kc + 1) * 128, :2 * D])

                                stage = [None, None]

                                def emit_sc(i):
                                    b, h = i // H, i % H
                                    cch, half = h // 2, (h % 2) * 64
                                    mask = mask0 if b == 0 else mask1
                                    sts = []
                                    for g2 in range(2):
                                        st = apsum.tile([128, 512], F32,
                                                        tag="st")
                                        for kk in range(2):
                                            kc = g2 * 2 + kk
                                            k0 = C * b + 128 * kc
                                            nc.tensor.matmul(
                                                st[:, kk * C:(kk + 1) * C],
                                                kts[cch][half:half + 64,
                                                         k0:k0 + 128],
                                                qts[cch][half:half + 64,
                                                         C * b:C * (b + 1)],
                                                start=True, stop=True)
                                        sts.append(st)
                                    pt = ptp.tile([128, 4 * C], BF16,
                                                  tag="pt")
                                    for g2 in range(2):
                                        nc.scalar.activation(
                                            pt[:, g2 * 512:(g2 + 1) * 512],
                                            sts[g2][:], AF.Exp,
                                            scale=float(1.0 / np.sqrt(DH)))
                                    nc.vector.tensor_mul(pt[:], pt[:], mask[:])
                                    if i == 2:
                                        tap("dbg_pt", pt)
                                    stage[i % 2] = pt

                                def emit_pv(i):
                                    b, h = i // H, i % H
                                    cch, half = h // 2, (h % 2) * 64
                                    pt = stage[i % 2]
                                    po = apsum.tile([128, C], F32,
                                                    tag="po", bufs=2)
                                    for kc in range(4):
                                        nc.tensor.matmul(
                                            po[:],
                                            vts[2 * b + kc][:,
                                                h * 128:(h + 1) * 128],
                                            pt[:, kc * C:(kc + 1) * C],
                                            start=(kc == 0), stop=(kc == 3))
                                    if i == 2 and taps:
                                        pc = rp.tile([128, C], F32,
                                                     tag="dpo", bufs=1)
                                        nc.vector.tensor_copy(pc[:], po[:])
                                        tap("dbg_po", pc)
                                        tap("dbg_den", pc[0:1, :])
                                    rrow = rp.tile([1, C], F32, tag="rr")
                                    nc.vector.reciprocal_approx_fast(
                                        rrow[:], po[0:1, :])
                                    rrowr = rp.tile([1, C], F32R, tag="rrr")
                                    nc.vector.tensor_copy(rrowr[:], rrow[:])
                                    rb = apsum.tile([64, C], F32, tag="rb",
                                                    bufs=2)
                                    nc.tensor.matmul(rb[:],
                                                     ones_row_r[:, :64],
                                                     rrowr[:],
                                                     start=True, stop=True)
                                    rs = rp.tile([64, C], BF16, tag="r64")
                                    nc.scalar.activation(
                                        rs[:], rb[:], AF.Copy, bias=0.0)
                                    if i == 2 and taps:
                                        tap("dbg_rr", rrow)
                                        rc = rp.tile([64, C], F32, tag="drb",
                                                     bufs=1)
                                        nc.vector.tensor_copy(rc[:], rb[:])
                                        tap("dbg_rb", rc)
                                    nc.vector.tensor_mul(
                                        attns[cch][half:half + 64,
                                                   C * b:C * (b + 1)],
                                        po[64:128, :], rs[:])

                                for i in range(2 * H + 1):
                                    if i < 2 * H:
                                        emit_sc(i)
                                    if i - 1 >= 0:
                                        emit_pv(i - 1)
                        tap("dbg_at", attns[0])
                    # act0 closed: xn/kt/v/qt freed

                    # ---- W_O + gated fusion --------------------------------
                    with tc.tile_pool(name="wfus", bufs=10) as wf, \
                         tc.tile_pool(name="spsum", bufs=4, space="PSUM") as sp:
                        xms = [persist(xm_pool, [128, TOWN], f"xm{c}", F32R)
                               for c in range(8)]
                        for mg in range(2):
                            wos = [load_w_pool(wf, nc, d_wo, kc, mg * 512, 512)
                                   for kc in range(8)]
                            for j in range(4):
                                mc = mg * 4 + j
                                ps = sp.tile([128, 512], F32, tag="s")
                                for kc in range(8):
                                    nc.tensor.matmul(
                                        ps[:],
                                        wos[kc][:, j * 128:(j + 1) * 128],
                                        attns[kc][:],
                                        start=(kc == 0), stop=(kc == 7))
                                xm = xms[mc]
                                tdel = wf.tile([128, TOWN], BF16, tag="td",
                                               bufs=3)
                                nc.vector.scalar_tensor_tensor(
                                    tdel[:], ps[:], bias_col(BC_O + mc),
                                    ys[mc][:], op0=OP.add, op1=OP.subtract)
                                nc.vector.tensor_mul(tdel[:], tdel[:],
                                                     gs[mc][:])
                                nc.vector.tensor_add(xm[:], tdel[:],
                                                     yxos[mc][:])
                        tap("dbg_xm", xms[0])
                # act1 closed: g/attn/drive/states/y freed

                # ====== phase group B: LN2 + MLP ==============================
                with tc.tile_pool(name="xn2p", bufs=1) as xn2p:
                    with tc.tile_pool(name="lnst2", bufs=1) as lnst2, \
                         tc.tile_pool(name="ln2psum", bufs=2,
                                      space="PSUM") as lnp2:
                        xn2s = layernorm(xms, TOWN, lnst2, lnp2, xn2p, "h",
                                         ones_lnr, F32R)
                    tap("dbg_xn2", xn2s[0])
                    with tc.tile_pool(name="hTp", bufs=1) as hTp, \
                         tc.tile_pool(name="wmlp", bufs=12) as wm:
                        hts = [persist(hTp, [128, TOWN], f"ht{c}", BF16)
                               for c in range(32)]
                        with tc.tile_pool(name="m1psum", bufs=4,
                                          space="PSUM") as mp1:
                            for mg in range(8):
                                if mg < 4:
                                    stat = [
                                        (w1ts[kc], (mg % 4) * 512)
                                        for kc in range(8)]
                                else:
                                    ws = [load_w_pool(wm, nc, d_w1, kc,
                                                      mg * 512, 512)
                                          for kc in range(8)]
                                    stat = [(ws[kc], 0) for kc in range(8)]
                                for j in range(4):
                                    mc = mg * 4 + j
                                    ps = mp1.tile([128, 512], F32, tag="m")
                                    for kc in range(8):
                                        wt, off = stat[kc]
                                        nc.tensor.matmul(
                                            ps[:],
                                            wt[:, off + j * 128:
                                               off + (j + 1) * 128],
                                            xn2s[kc][:],
                                            start=(kc == 0), stop=(kc == 7))
                                    nc.scalar.activation(
                                        hts[mc][:], ps[:], AF.Gelu,
                                        bias=bias_col(BC_B1 + mc))
                        tap("dbg_h", hts[0])
                        with tc.tile_pool(name="m2psum", bufs=1,
                                          space="PSUM") as mp2, \
                             tc.tile_pool(name="outp", bufs=3) as outp:
                            pss = [mp2.tile([128, 512], F32, tag=f"o{mc}",
                                            name=f"o{mc}", bufs=1)
                                   for mc in range(8)]
                            for kc in range(32):
                                w2r = wm.tile([128, 1024], BF16, tag="w2",
                                              bufs=6)
                                nc.sync.dma_start(
                                    w2r[:], d_w2[kc * 128:(kc + 1) * 128, :])
                                for mc in range(8):
                                    nc.tensor.matmul(
                                        pss[mc][:],
                                        w2r[:, mc * 128:(mc + 1) * 128],
                                        hts[kc][:],
                                        start=(kc == 0), stop=(kc == 31))
                            for mc in range(8):
                                oc = outp.tile([128, TOWN], F32, tag="oc")
                                nc.vector.scalar_tensor_tensor(
                                    oc[:], pss[mc][:], bias_col(BC_B2 + mc),
                                    xms[mc][:], op0=OP.add, op1=OP.add)
                                nc.sync.dma_start(
                                    d_out[mc * 128:(mc + 1) * 128, :], oc[:])

    nc.compile()
    return nc


def load_w_pool(pool, nc, dram, kc, m0, mw):
    w = pool.tile([128, 512], BF16, tag="w")
    nc.sync.dma_start(w[:, :mw], dram[kc * 128:(kc + 1) * 128, m0:m0 + mw])
    return w


def _make_masks():
    qi = np.arange(C)[:, None]
    kk = np.arange(2 * C)[None, :]
    band = (kk > qi) & (kk <= qi + C)
    first = band & (kk >= C)

    def pack(m):                       # [C, 2C] -> [128, 4*C] k-chunk-major
        mt = m.T.astype(np.float32)    # [2C, C]
        return np.ascontiguousarray(
            mt.reshape(4, 128, C).transpose(1, 0, 2).reshape(128, 4 * C)
        ).astype(NPBF)

    return pack(first), pack(band)


def _prep_inputs(x, ln1_g, ln1_b, ln2_g, ln2_b, W_qkv, W_O, b_O, W_ug, b_ug,
                 B_w, A, C_w, mlp_W1, mlp_b1, mlp_W2, mlp_b2):
    f = np.float32
    g1 = np.asarray(ln1_g, f)
    b1 = np.asarray(ln1_b, f)
    W_qkv = np.asarray(W_qkv, f)
    W_qkv_e = g1[:, None] * W_qkv
    b_qkv_e = b1 @ W_qkv
    W_ug = np.asarray(W_ug, f)
    B_w = np.asarray(B_w, f)
    b_ug = np.asarray(b_ug, f)
    W_drive_raw = B_w + W_ug[:, :S]
    W_drive_e = g1[:, None] * W_drive_raw
    b_drive_e = b1 @ W_drive_raw + b_ug[:S]
    W_gate_e = g1[:, None] * W_ug[:, S:]
    b_gate_e = b1 @ W_ug[:, S:] + b_ug[S:]
    g2 = np.asarray(ln2_g, f)
    b2l = np.asarray(ln2_b, f)
    mlp_W1 = np.asarray(mlp_W1, f)
    W1_e = g2[:, None] * mlp_W1
    b1_e = b2l @ mlp_W1 + np.asarray(mlp_b1, f)

    biaspack = np.zeros((128, NBC), f)
    biaspack[:, BC_A] = np.asarray(A, f)
    biaspack[:, BC_QKV:BC_QKV + 24] = b_qkv_e.reshape(24, 128).T
    biaspack[:, BC_GATE:BC_GATE + 8] = b_gate_e.reshape(8, 128).T
    biaspack[:, BC_DRIVE] = b_drive_e
    biaspack[:, BC_O:BC_O + 8] = np.asarray(b_O, f).reshape(8, 128).T
    biaspack[:, BC_B1:BC_B1 + 32] = b1_e.reshape(32, 128).T
    biaspack[:, BC_B2:BC_B2 + 8] = np.asarray(mlp_b2, f).reshape(8, 128).T
    vbias = np.ascontiguousarray(
        np.broadcast_to(b_qkv_e[2 * D:].reshape(1, D), (128, D))).astype(NPBF)

    m_first, m_band = _make_masks()
    xTfull = np.ascontiguousarray(np.asarray(x, f)[0].T)

    shared = {
        "biaspack": biaspack, "vbias": vbias,
        "wqkv": np.ascontiguousarray(W_qkv_e.astype(NPBF)),
        "wgate": np.ascontiguousarray(W_gate_e.astype(NPBF)),
        "wdrive": np.ascontiguousarray(W_drive_e.astype(NPBF)),
        "wo": np.ascontiguousarray(np.asarray(W_O, f).astype(NPBF)),
        "cw": np.ascontiguousarray(np.asarray(C_w, f)),
        "w1": np.ascontiguousarray(W1_e.astype(NPBF)),
        "w2": np.ascontiguousarray(np.asarray(mlp_W2, f).astype(NPBF)),
        "mask1": m_band,
    }
    in_maps = []
    for i in range(NCORES):
        t0 = i * TOWN
        xT = np.zeros((D, TLOC), f)
        lo = max(0, t0 - HALO)
        xT[:, HALO - (t0 - lo):HALO] = xTfull[:, lo:t0]
        xT[:, HALO:] = xTfull[:, t0:t0 + TOWN]
        m0 = m_first if i == 0 else m_band
        in_maps.append({**shared,
                        "xT": np.ascontiguousarray(xT.astype(NPBF)),
                        "xown": np.ascontiguousarray(
                            xTfull[:, t0:t0 + TOWN]),
                        "mask0": m0})
    return in_maps


_CACHED_NC = None


def get_nc():
    global _CACHED_NC
    if _CACHED_NC is None:
        _CACHED_NC = build_program()
    return _CACHED_NC


def kernel(**inputs):
    nc = get_nc()
    in_maps = _prep_inputs(**inputs)
    res = bass_utils.run_bass_kernel_spmd(nc, in_maps,
                                          core_ids=list(range(NCORES)))
    out = np.empty((1, T, D), np.float32)
    for i in range(NCORES):
        out[0, i * TOWN:(i + 1) * TOWN, :] = res.results[i]["outT"].T
    return out
